# revision 7
# baseline (speedup 1.0000x reference)
"""Trainium2 Bass kernel for a dense transformer block (B=2, T=2048, D=1024, H=16).

Sharding: 8 cores; core c handles batch b=c//4, query-token block r=c%4
(512 tokens). Each core computes LN1, projects K/V for its own tokens,
AllGathers K/V across its 4-core batch group, then runs full non-causal
attention for its 512 query rows over all 2048 keys, o-proj + residual,
LN2, and the FFN — all with activations kept feature-major [feat, token].
Matmuls keep f32r moving operands (activations); weights are bf16
stationary operands (full PE rate either way).

Host<->device traffic is the wall-clock bottleneck (the axon tunnel moves
only a few MB/s per stream), so:
  - weights are row-sharded 8x on the host (f32r pack + bf16 w2; the HW
    compiler rejects mixed 32/16-bit matmul operands, so the f32r
    activation path needs f32r weights) and AllGathered to full
    [D,8192]/[FF,D] DRAM copies on device;
  - x ships bf16 feature-major per core (1 MB/core), converted to f32r
    in SBUF;
  - the output returns bf16 (1 MB/core);
  - device-resident weight arrays and the jitted executable are cached
    across calls (weights keyed by a blake2b content hash), so warm calls
    only move x up and the output down.
"""
import hashlib
import os

import numpy as np
import ml_dtypes

import jax
from jax.experimental.shard_map import shard_map
from jax.sharding import Mesh, NamedSharding, PartitionSpec

import concourse.bass as bass  # noqa: F401
import concourse.mybir as mybir
import concourse.tile as tile
from concourse import bacc
from concourse.tile import add_dep_helper
from concourse.bass2jax import _bass_exec_p, install_neuronx_cc_hook, partition_id_tensor

F32 = mybir.dt.float32
F32R = mybir.dt.float32r
BF16 = mybir.dt.bfloat16
AF = mybir.ActivationFunctionType
ALU = mybir.AluOpType

B, T, D, H = 2, 2048, 1024, 16
HS = D // H  # 64
FF = 4 * D
TLOC = 512
NCORES = 8
RG = [[0, 1, 2, 3], [4, 5, 6, 7]]
RG8 = [[0, 1, 2, 3, 4, 5, 6, 7]]
EPS = 1e-5
WCOLS = 4 * D + FF  # 8192: [wq|wk|wv|wo|w1]

BF16_NP = ml_dtypes.bfloat16

_STATE: dict = {}
# zeros handling for the donated output buffers:
#   donate: fresh np zeros each call, donated (exact run_bass_via_pjrt semantics)
#   none:   no output operands at all (kernel writes every element of outT)
_ZMODE = os.environ.get("ZMODE", "donate")


def _build():
    nc = bacc.Bacc("TRN2", target_bir_lowering=False, debug=False, num_devices=NCORES)

    xTb = nc.declare_dram_parameter("xTb", [D, TLOC], BF16, isOutput=False)
    wsh = nc.declare_dram_parameter("wsh", [128, WCOLS], F32R, isOutput=False)
    w2sh = nc.declare_dram_parameter("w2sh", [512, D], BF16, isOutput=False)
    gb1 = nc.declare_dram_parameter("gb1", [8, 2, 128], F32R, isOutput=False)
    gb2 = nc.declare_dram_parameter("gb2", [8, 2, 128], F32R, isOutput=False)
    bo_r = nc.declare_dram_parameter("bo_r", [8, 128], F32, isOutput=False)
    b1_r = nc.declare_dram_parameter("b1_r", [32, 128], F32, isOutput=False)
    b2_r = nc.declare_dram_parameter("b2_r", [8, 128], F32, isOutput=False)
    outT = nc.declare_dram_parameter("outT", [D, TLOC], BF16, isOutput=True)

    wstg = nc.dram_tensor("wstg", [128, WCOLS], F32R)
    w2stg = nc.dram_tensor("w2stg", [512, D], BF16)
    wfull = nc.dram_tensor("wfull", [D, WCOLS], F32R)
    w2full = nc.dram_tensor("w2full", [FF, D], BF16)
    agk_in = nc.dram_tensor("agk_in", [D, TLOC], F32R)
    agk_out = nc.dram_tensor("agk_out", [4 * D, TLOC], F32R)
    agv_in = nc.dram_tensor("agv_in", [TLOC, H * (HS + 1)], F32R)
    agv_out = nc.dram_tensor("agv_out", [4 * TLOC, H * (HS + 1)], F32R)

    # column offsets into wfull
    OQ, OK, OV, OO, O1 = 0, D, 2 * D, 3 * D, 4 * D

    with tile.TileContext(nc) as tc:
        from contextlib import ExitStack

        ctx = ExitStack()
        big = ctx.enter_context(tc.tile_pool(name="big", bufs=8))
        h3p = ctx.enter_context(tc.tile_pool(name="h3p", bufs=32))
        wp = ctx.enter_context(tc.tile_pool(name="wp", bufs=4))
        kfp = ctx.enter_context(tc.tile_pool(name="kfp", bufs=6))
        vfp = ctx.enter_context(tc.tile_pool(name="vfp", bufs=6))
        ptp = ctx.enter_context(tc.tile_pool(name="ptp", bufs=4))
        stg = ctx.enter_context(tc.tile_pool(name="stg", bufs=2))
        sc = ctx.enter_context(tc.tile_pool(name="sc", bufs=1))
        xbp = ctx.enter_context(tc.tile_pool(name="xbp", bufs=2))
        pp = ctx.enter_context(tc.tile_pool(name="pp", bufs=4, space="PSUM"))

        def pslot(name):
            return pp.tile([128, 2 * TLOC], F32, tag="ps", name=name)

        # weight AllGather first — overlaps with LN1 on the compute engines.
        # Collectives cannot read IO tensors, so stage the param shards into
        # scratch DRAM with a DMA first.
        d_w = nc.sync.dma_start(out=wstg.ap(), in_=wsh.ap())
        d_w2 = nc.sync.dma_start(out=w2stg.ap(), in_=w2sh.ap())
        cc_w = nc.gpsimd.collective_compute(
            "AllGather", ALU.bypass, replica_groups=RG8,
            ins=[wstg.ap().opt()], outs=[wfull.ap().opt()],
        )
        add_dep_helper(cc_w.ins, d_w.ins, reason="AG after param stage")

        def wload(name, col_off, k, ncols=D):
            wt = wp.tile([128, ncols], F32R, tag="wmat", name=name)
            d = nc.sync.dma_start(
                out=wt,
                in_=wfull[128 * k : 128 * (k + 1), col_off : col_off + ncols],
            )
            add_dep_helper(d.ins, cc_w.ins, reason="W read after AG")
            return wt

        ones_kf = sc.tile([128, 1], F32, tag="ones_kf")
        nc.vector.memset(ones_kf, 1.0)
        ones_k = sc.tile([128, 1], F32R, tag="ones_k")
        nc.vector.tensor_copy(ones_k, ones_kf)
        ones16 = sc.tile([128, 16], F32R, tag="ones16")
        nc.vector.tensor_copy(ones16, ones_kf.to_broadcast([128, 16]))
        ones64f = sc.tile([1, HS], F32, tag="ones64f")
        nc.vector.memset(ones64f, 1.0)
        ones64 = sc.tile([1, HS], F32R, tag="ones64")
        nc.vector.tensor_copy(ones64, ones64f)
        eps_t = sc.tile([1, 1], F32, tag="eps")
        nc.vector.memset(eps_t, EPS)

        xt = []
        for k in range(8):
            xb = xbp.tile([128, TLOC], BF16, tag="xb")
            nc.sync.dma_start(out=xb, in_=xTb[128 * k : 128 * (k + 1), :])
            t = big.tile([128, TLOC], F32R, tag="xt", name=f"xt{k}")
            nc.vector.tensor_copy(t, xb)
            xt.append(t)

        def layer_norm(src_tiles, gb_dram, ln_id):
            st_slot = pslot(f"lnstat{ln_id}")
            ps_s1 = st_slot[0:1, 0:TLOC]
            ps_s2 = st_slot[0:1, TLOC : 2 * TLOC]
            for k in range(8):
                nc.tensor.matmul(ps_s1, ones_k, src_tiles[k],
                                 start=(k == 0), stop=(k == 7))
            for k in range(8):
                xsq = stg.tile([128, TLOC], F32R, tag="xsq")
                nc.vector.tensor_mul(xsq, src_tiles[k], src_tiles[k])
                nc.tensor.matmul(ps_s2, ones_k, xsq,
                                 start=(k == 0), stop=(k == 7))
            mu = sc.tile([1, TLOC], F32, tag="mu")
            nc.scalar.mul(mu, ps_s1, 1.0 / D)
            musq = sc.tile([1, TLOC], F32, tag="musq")
            nc.vector.tensor_mul(musq, mu, mu)
            var = sc.tile([1, TLOC], F32, tag="var")
            nc.vector.scalar_tensor_tensor(
                out=var, in0=ps_s2, scalar=1.0 / D, in1=musq,
                op0=ALU.mult, op1=ALU.subtract,
            )
            sd = sc.tile([1, TLOC], F32, tag="sd")
            nc.scalar.activation(sd, var, AF.Sqrt, bias=eps_t[0:1, :])
            rstd_f = sc.tile([1, TLOC], F32, tag="rstd_f")
            nc.vector.reciprocal(rstd_f, sd)
            rstd = sc.tile([1, TLOC], F32R, tag="rstd")
            nc.vector.tensor_copy(rstd, rstd_f)
            rhs2f = sc.tile([2, TLOC], F32, tag="rhs2f")
            nc.vector.memset(rhs2f, 1.0)
            nc.vector.tensor_mul(rhs2f[0:1, :], mu, rstd_f)
            nc.vector.tensor_scalar_mul(rhs2f[0:1, :], rhs2f[0:1, :], -1.0)
            rhs2 = sc.tile([2, TLOC], F32R, tag="rhs2")
            nc.vector.tensor_copy(rhs2, rhs2f)
            out_tiles = []
            for m in range(8):
                gb = sc.tile([2, 128], F32R, tag="gb")
                nc.sync.dma_start(out=gb, in_=gb_dram[m, :, :])
                bc = pslot(f"lnbc{ln_id}_{m}")
                ps_A = bc[:, 0:TLOC]
                ps_C = bc[:, TLOC : 2 * TLOC]
                nc.tensor.matmul(ps_A, gb[0:1, :], rstd, start=True, stop=True)
                nc.tensor.matmul(ps_C, gb, rhs2, start=True, stop=True)
                h = big.tile([128, TLOC], F32R, tag="ht", name=f"ht{ln_id}_{m}")
                nc.vector.tensor_mul(h, src_tiles[m], ps_A)
                nc.vector.tensor_add(h, h, ps_C)
                out_tiles.append(h)
            return out_tiles

        h1t = layer_norm(xt, gb1, "1")

        # ---- K projection -> AllGather ----
        slots = [pslot(f"psK{i}") for i in range(4)]
        psK = [slots[i // 2][:, TLOC * (i % 2) : TLOC * (i % 2 + 1)]
               for i in range(8)]
        for k in range(8):
            wt = wload(f"wtk{k}", OK, k)
            for m in range(8):
                nc.tensor.matmul(
                    psK[m], wt[:, 128 * m : 128 * (m + 1)], h1t[k],
                    start=(k == 0), stop=(k == 7),
                )
        for m in range(8):
            ksb = stg.tile([128, TLOC], F32R, tag="ktsb")
            nc.vector.tensor_copy(ksb, psK[m])
            nc.sync.dma_start(out=agk_in[128 * m : 128 * (m + 1), :], in_=ksb)
        del psK, slots
        cc_k = nc.gpsimd.collective_compute(
            "AllGather", ALU.bypass, replica_groups=RG,
            ins=[agk_in.ap().opt()], outs=[agk_out.ap().opt()],
        )

        # ---- V projection (token-major, ones col) -> AllGather ----
        slots = [pslot(f"psV{i}") for i in range(4)]
        psV = [slots[i // 2][:, TLOC * (i % 2) : TLOC * (i % 2 + 1)]
               for i in range(8)]
        for k in range(8):
            wt = wload(f"wtv{k}", OV, k)
            for t in range(4):
                lhs = h1t[k][:, 128 * t : 128 * (t + 1)]
                nc.tensor.matmul(psV[2 * t], lhs, wt[:, 0:512],
                                 start=(k == 0), stop=(k == 7))
                nc.tensor.matmul(psV[2 * t + 1], lhs, wt[:, 512:1024],
                                 start=(k == 0), stop=(k == 7))
        for t in range(4):
            vsb = stg.tile([128, H * (HS + 1)], F32R, tag="vsb")
            vsb3 = vsb.rearrange("p (h w) -> p h w", w=HS + 1)
            nc.vector.tensor_copy(
                vsb3[:, 0:8, 0:HS],
                psV[2 * t].rearrange("p (h w) -> p h w", w=HS),
            )
            nc.vector.tensor_copy(
                vsb3[:, 8:16, 0:HS],
                psV[2 * t + 1].rearrange("p (h w) -> p h w", w=HS),
            )
            nc.vector.tensor_copy(
                vsb3[:, :, HS : HS + 1],
                ones16.rearrange("p (h o) -> p h o", o=1),
            )
            nc.sync.dma_start(out=agv_in[128 * t : 128 * (t + 1), :], in_=vsb)
        del psV, slots
        cc_v = nc.gpsimd.collective_compute(
            "AllGather", ALU.bypass, replica_groups=RG,
            ins=[agv_in.ap().opt()], outs=[agv_out.ap().opt()],
        )
        # w2 gather is only needed by the FFN down-projection at the very
        # end; issue it after the K/V gathers so it doesn't delay them on
        # the serial gpsimd collective queue.
        cc_w2 = nc.gpsimd.collective_compute(
            "AllGather", ALU.bypass, replica_groups=RG8,
            ins=[w2stg.ap().opt()], outs=[w2full.ap().opt()],
        )
        add_dep_helper(cc_w2.ins, d_w2.ins, reason="AG after param stage")

        # ---- Q projection (kept in SBUF) ----
        slots = [pslot(f"psQ{i}") for i in range(4)]
        psQ = [slots[i // 2][:, TLOC * (i % 2) : TLOC * (i % 2 + 1)]
               for i in range(8)]
        for k in range(8):
            wt = wload(f"wtq{k}", OQ, k)
            for m in range(8):
                nc.tensor.matmul(
                    psQ[m], wt[:, 128 * m : 128 * (m + 1)], h1t[k],
                    start=(k == 0), stop=(k == 7),
                )
        qt = []
        for m in range(8):
            q = big.tile([128, TLOC], F32R, tag="qx", name=f"qt{m}")
            nc.vector.tensor_copy(q, psQ[m])
            qt.append(q)
        del psQ, slots

        # ---- attention, one head pair at a time ----
        ot = []
        for hp in range(8):
            kf = []
            vf = []
            for r in range(4):
                kt_ = kfp.tile([128, TLOC], F32R, tag="kf")
                d = nc.sync.dma_start(
                    out=kt_,
                    in_=agk_out[1024 * r + 128 * hp : 1024 * r + 128 * (hp + 1), :],
                )
                add_dep_helper(d.ins, cc_k.ins, reason="K read after AG")
                kf.append(kt_)
                vt_ = vfp.tile([128, 4, 2 * (HS + 1)], F32R, tag="vf")
                d = nc.sync.dma_start(
                    out=vt_,
                    in_=agv_out[
                        TLOC * r : TLOC * (r + 1),
                        130 * hp : 130 * (hp + 1),
                    ].rearrange("(c p) w -> p c w", p=128),
                )
                add_dep_helper(d.ins, cc_v.ins, reason="V read after AG")
                vf.append(vt_)

            oslot = pslot(f"psO{hp}")
            psOA = oslot[0 : HS + 1, 0:TLOC]
            psOB = oslot[0 : HS + 1, TLOC : 2 * TLOC]
            qA = qt[hp][0:HS, :]
            qB = qt[hp][HS:128, :]
            for scp in range(8):
                psSA = pslot(f"psSA{hp}_{scp}")
                psSB = pslot(f"psSB{hp}_{scp}")
                for j in range(2):
                    s_chunk = 2 * scp + j
                    r, c = divmod(s_chunk, 4)
                    lhsA = kf[r][0:HS, 128 * c : 128 * (c + 1)]
                    lhsB = kf[r][HS:128, 128 * c : 128 * (c + 1)]
                    nc.tensor.matmul(
                        psSA[:, TLOC * j : TLOC * (j + 1)], lhsA, qA,
                        start=True, stop=True, tile_position=(0, 0),
                    )
                    nc.tensor.matmul(
                        psSB[:, TLOC * j : TLOC * (j + 1)], lhsB, qB,
                        start=True, stop=True, tile_position=(64, 0),
                    )
                ptA = ptp.tile([128, 2 * TLOC], F32R, tag="pt")
                nc.scalar.activation(ptA, psSA, AF.Exp, scale=HS**-0.5)
                ptB = ptp.tile([128, 2 * TLOC], F32R, tag="pt")
                nc.scalar.activation(ptB, psSB, AF.Exp, scale=HS**-0.5)
                for j in range(2):
                    s_chunk = 2 * scp + j
                    r, c = divmod(s_chunk, 4)
                    nc.tensor.matmul(
                        psOA, vf[r][:, c, 0 : HS + 1],
                        ptA[:, TLOC * j : TLOC * (j + 1)],
                        start=(s_chunk == 0), stop=(s_chunk == 15),
                    )
                    nc.tensor.matmul(
                        psOB, vf[r][:, c, HS + 1 : 2 * (HS + 1)],
                        ptB[:, TLOC * j : TLOC * (j + 1)],
                        start=(s_chunk == 0), stop=(s_chunk == 15),
                    )
            o = big.tile([128, TLOC], F32R, tag="ot", name=f"ot{hp}")
            rbslot = pslot(f"psRb{hp}")
            for half, psO in ((0, psOA), (1, psOB)):
                rec_f = sc.tile([1, TLOC], F32, tag=f"rec_f{half}")
                nc.vector.reciprocal(rec_f, psO[HS : HS + 1, :])
                rec = sc.tile([1, TLOC], F32R, tag=f"rec{half}")
                nc.vector.tensor_copy(rec, rec_f)
                psRb = rbslot[0:HS, TLOC * half : TLOC * (half + 1)]
                nc.tensor.matmul(psRb, ones64, rec, start=True, stop=True)
                rb_sb = stg.tile([HS, TLOC], F32, tag=f"rb{half}")
                nc.vector.tensor_copy(rb_sb, psRb)
                nc.vector.tensor_mul(
                    o[HS * half : HS * (half + 1), :], psO[0:HS, :], rb_sb
                )
            ot.append(o)

        # ---- o-proj + residual ----
        slots = [pslot(f"psO2{i}") for i in range(4)]
        psO2 = [slots[i // 2][:, TLOC * (i % 2) : TLOC * (i % 2 + 1)]
                for i in range(8)]
        for k in range(8):
            wt = wload(f"wto{k}", OO, k)
            for m in range(8):
                nc.tensor.matmul(
                    psO2[m], wt[:, 128 * m : 128 * (m + 1)], ot[k],
                    start=(k == 0), stop=(k == 7),
                )
        x2t = []
        for m in range(8):
            bo_sc = sc.tile([128, 1], F32, tag="bo_sc")
            nc.sync.dma_start(
                out=bo_sc, in_=bo_r[m : m + 1, :].rearrange("o p -> p o")
            )
            x2 = big.tile([128, TLOC], F32R, tag="qx", name=f"x2t{m}")
            nc.vector.scalar_tensor_tensor(
                out=x2, in0=psO2[m], scalar=bo_sc, in1=xt[m],
                op0=ALU.add, op1=ALU.add,
            )
            x2t.append(x2)
        del psO2, slots

        h2t = layer_norm(x2t, gb2, "2")

        # ---- FFN up (+relu, bf16 out) ----
        h3 = []
        for mg in range(4):
            slots = [pslot(f"psF{mg}_{i}") for i in range(4)]
            psF = [slots[i // 2][:, TLOC * (i % 2) : TLOC * (i % 2 + 1)]
                   for i in range(8)]
            for k in range(8):
                wt = wload(f"wt1_{mg}_{k}", O1 + D * mg, k)
                for ml in range(8):
                    nc.tensor.matmul(
                        psF[ml], wt[:, 128 * ml : 128 * (ml + 1)], h2t[k],
                        start=(k == 0), stop=(k == 7),
                    )
            for ml in range(8):
                row = 8 * mg + ml
                b1sc = sc.tile([128, 1], F32, tag="b1sc")
                nc.sync.dma_start(
                    out=b1sc, in_=b1_r[row : row + 1, :].rearrange("o p -> p o")
                )
                h3_t = h3p.tile([128, TLOC], BF16, tag="h3", name=f"h3_{row}")
                nc.scalar.activation(h3_t, psF[ml], AF.Relu, bias=b1sc[:, 0:1])
                h3.append(h3_t)
            del psF, slots

        # ---- FFN down (bf16) + residual + out ----
        slots = [pslot(f"psY{i}") for i in range(4)]
        psY = [slots[i // 2][:, TLOC * (i % 2) : TLOC * (i % 2 + 1)]
               for i in range(8)]
        for k2 in range(32):
            wt = wp.tile([128, D], BF16, tag="wmat", name=f"wt2_{k2}")
            d = nc.sync.dma_start(
                out=wt, in_=w2full[128 * k2 : 128 * (k2 + 1), :]
            )
            add_dep_helper(d.ins, cc_w2.ins, reason="W2 read after AG")
            for m in range(8):
                nc.tensor.matmul(
                    psY[m], wt[:, 128 * m : 128 * (m + 1)], h3[k2],
                    start=(k2 == 0), stop=(k2 == 31),
                )
        for m in range(8):
            b2sc = sc.tile([128, 1], F32, tag="b2sc")
            nc.sync.dma_start(
                out=b2sc, in_=b2_r[m : m + 1, :].rearrange("o p -> p o")
            )
            fin = stg.tile([128, TLOC], BF16, tag="fin")
            nc.vector.scalar_tensor_tensor(
                out=fin, in0=psY[m], scalar=b2sc, in1=x2t[m],
                op0=ALU.add, op1=ALU.add,
            )
            nc.sync.dma_start(out=outT[128 * m : 128 * (m + 1), :], in_=fin)
        del psY, slots

        ctx.close()
    nc.finalize()
    return nc


def _ensure_exec():
    if "sharded" in _STATE:
        return
    install_neuronx_cc_hook()
    nc = _build()
    assert nc.dbg_addr is None

    partition_name = nc.partition_id_tensor.name if nc.partition_id_tensor else None
    in_names: list[str] = []
    out_names: list[str] = []
    out_avals: list = []
    for alloc in nc.m.functions[0].allocations:
        if not isinstance(alloc, mybir.MemoryLocationSet):
            continue
        name = alloc.memorylocations[0].name
        if alloc.kind == "ExternalInput":
            if name != partition_name:
                in_names.append(name)
        elif alloc.kind == "ExternalOutput":
            out_names.append(name)
            shape = tuple(alloc.tensor_shape)
            dtype = mybir.dt.np(alloc.dtype)
            out_avals.append(jax.core.ShapedArray(shape, dtype))
    n_params = len(in_names)
    n_outs = len(out_avals)

    in_names_full = list(in_names)
    if _ZMODE != "none":
        in_names_full.extend(out_names)
    if partition_name is not None:
        in_names_full.append(partition_name)

    def _body(*args):
        operands = list(args)
        if partition_name is not None:
            operands.append(partition_id_tensor())
        outs = _bass_exec_p.bind(
            *operands,
            out_avals=tuple(out_avals),
            in_names=tuple(in_names_full),
            out_names=tuple(out_names),
            lowering_input_output_aliases=(),
            sim_require_finite=True,
            sim_require_nnan=True,
            nc=nc,
        )
        return tuple(outs)

    devices = jax.devices()[:NCORES]
    assert len(devices) == NCORES
    mesh = Mesh(np.asarray(devices), ("core",))
    n_args = n_params + (n_outs if _ZMODE != "none" else 0)
    in_specs = (PartitionSpec("core"),) * n_args
    out_specs = (PartitionSpec("core"),) * n_outs
    donate = tuple(range(n_params, n_args)) if _ZMODE == "donate" else ()
    sharded = jax.jit(
        shard_map(
            _body, mesh=mesh, in_specs=in_specs, out_specs=out_specs, check_rep=False
        ),
        donate_argnums=donate,
        keep_unused=True,
    )
    _STATE.update(
        nc=nc, mesh=mesh, sharded=sharded, in_names=in_names,
        out_names=out_names, out_avals=out_avals, n_params=n_params,
        n_outs=n_outs,
        sharding=NamedSharding(mesh, PartitionSpec("core")),
    )


def _weight_hash(arrs):
    h = hashlib.blake2b(digest_size=16)
    for a in arrs:
        h.update(np.ascontiguousarray(a).view(np.uint8).data)
    return h.digest()


def _prep_weights(Wq, Wk, Wv, Wo, bo, W1, b1, W2, b2, ln1_g, ln1_b, ln2_g, ln2_b):
    key = _weight_hash([Wq, Wk, Wv, Wo, bo, W1, b1, W2, b2,
                        ln1_g, ln1_b, ln2_g, ln2_b])
    if _STATE.get("wkey") == key:
        return
    sh = _STATE["sharding"]
    wq2 = np.asarray(Wq, np.float32).transpose(1, 0, 2).reshape(D, D)
    wk2 = np.asarray(Wk, np.float32).transpose(1, 0, 2).reshape(D, D)
    wv2 = np.asarray(Wv, np.float32).transpose(1, 0, 2).reshape(D, D)
    wo2 = np.asarray(Wo, np.float32)
    w1a = np.asarray(W1, np.float32)
    pack = np.ascontiguousarray(np.concatenate([wq2, wk2, wv2, wo2, w1a], axis=1))
    w2b = np.asarray(W2, np.float32).astype(BF16_NP)
    gb1 = np.stack([np.asarray(ln1_g, np.float32).reshape(8, 128),
                    np.asarray(ln1_b, np.float32).reshape(8, 128)], axis=1)
    gb2 = np.stack([np.asarray(ln2_g, np.float32).reshape(8, 128),
                    np.asarray(ln2_b, np.float32).reshape(8, 128)], axis=1)
    dev = {
        "wsh": jax.device_put(pack, sh),
        "w2sh": jax.device_put(w2b, sh),
        "gb1": jax.device_put(np.tile(gb1, (NCORES, 1, 1)), sh),
        "gb2": jax.device_put(np.tile(gb2, (NCORES, 1, 1)), sh),
        "bo_r": jax.device_put(
            np.tile(np.asarray(bo, np.float32).reshape(8, 128), (NCORES, 1)), sh),
        "b1_r": jax.device_put(
            np.tile(np.asarray(b1, np.float32).reshape(32, 128), (NCORES, 1)), sh),
        "b2_r": jax.device_put(
            np.tile(np.asarray(b2, np.float32).reshape(8, 128), (NCORES, 1)), sh),
    }
    for v in dev.values():
        v.block_until_ready()
    _STATE["wdev"] = dev
    _STATE["wkey"] = key


def kernel(x, Wq, Wk, Wv, Wo, bo, W1, b1, W2, b2, ln1_g, ln1_b, ln2_g, ln2_b):
    _ensure_exec()
    _prep_weights(Wq, Wk, Wv, Wo, bo, W1, b1, W2, b2, ln1_g, ln1_b, ln2_g, ln2_b)

    x = np.asarray(x, np.float32)
    # per-core feature-major [D, TLOC] slices, concatenated core-major
    xg = (x.reshape(B, 4, TLOC, D).transpose(0, 1, 3, 2)
          .astype(BF16_NP).reshape(NCORES * D, TLOC))

    arrs = dict(_STATE["wdev"])
    arrs["xTb"] = xg
    args = [arrs[n] for n in _STATE["in_names"]]
    if _ZMODE != "none":
        for aval in _STATE["out_avals"]:
            shape = (NCORES * aval.shape[0],) + tuple(aval.shape[1:])
            args.append(np.zeros(shape, aval.dtype))
    out_arrs = _STATE["sharded"](*args)

    og = np.asarray(out_arrs[0])  # [NCORES*D, TLOC] bf16
    out = (og.reshape(B, 4, D, TLOC).transpose(0, 1, 3, 2)
           .astype(np.float32).reshape(B, T, D))
    return out
